# revision 2
# baseline (speedup 1.0000x reference)
"""Trainium2 Bass kernel for a dense transformer block (pre-LN, 8-head causal
attention + FFN), data-parallel over batch across 8 NeuronCores.

v4 over v3:
  * The six big GEMM families (Q, K, V, proj, W1, W2) run in fp8-e4m3
    DoubleRow mode: weights host-scaled by 64 (folded back out via the
    exp scale, the V copy scale, and scalar_tensor_tensor residual
    adds), activations h/h2/attn/relu quantized on the fly by the DVE/
    ACT op that produces them.  K=384 contractions run as one DoubleRow
    matmul (256) plus one plain fp8 matmul (128) in the same PSUM
    accumulation group -- no zero-padding.
  * LN mean-square on DVE (same-AP multiply) and both LN Sqrt ops
    clustered before QKV so the ACT function table swaps once per
    group instead of four times.
  * Attention and FFN interleaved per head pair; partition broadcasts
    on GPSIMD (proxy library); PSUM 8 banks as ps(2) + pair_ps(2) +
    score(2) + stats(2).
"""

import ml_dtypes
import numpy as np

import concourse.bass as bass
import concourse.mybir as mybir
import concourse.tile as tile
from concourse import bacc
from concourse import library_config
from concourse.bass_utils import run_bass_kernel_spmd

F32 = mybir.dt.float32
F16 = mybir.dt.float16
F8 = mybir.dt.float8e4
I32 = mybir.dt.int32
NPF8 = ml_dtypes.float8_e4m3
SCL = 64.0           # fp8 weight pre-scale (kept out of subnormals)
RSCL = 1.0 / SCL

# Model dims
B, T, C = 128, 256, 384
H, HS = 8, 48
FF = 4 * C           # 1536
EPS = 1e-5

# Sharding / tiling
NCORES = 8
NB = B // NCORES     # 16 sequences per core
TOK = NB * T         # 4096 tokens per core
P = 128
CCH = C // P         # 3 c-chunks
FCH = FF // P        # 12 ffn chunks
DPAD = 512           # q/k head-padded dim (4 tiles x 2 heads x 64)
QMT = DPAD // P      # 4
NPAIR = H // 2       # 4 head pairs
PW = 128             # [1a, va(48), z(15), 1b, vb(48), z(15)] per pair
VW = NPAIR * PW      # 512 augmented v width
GT = 512             # tokens per group (2 sequences)
NG = TOK // GT       # 8 groups
GTT = GT // P        # 4 token tiles per group
ISCALE = float(HS) ** -0.5
RC = 1.0 / C


def _build_program(flags):
    nc = bacc.Bacc(None, target_bir_lowering=False, debug=False)

    xt_d = nc.dram_tensor("xt", [C, TOK], F16, kind="ExternalInput").ap()
    wq_d = nc.dram_tensor("wq", [CCH, P, DPAD], F8, kind="ExternalInput").ap()
    wk_d = nc.dram_tensor("wk", [CCH, P, DPAD], F8, kind="ExternalInput").ap()
    wv_d = nc.dram_tensor("wv", [CCH, P, C], F8, kind="ExternalInput").ap()
    wp_d = nc.dram_tensor("wp", [QMT, P, C], F8, kind="ExternalInput").ap()
    w1_d = nc.dram_tensor("w1", [CCH, P, FF], F8, kind="ExternalInput").ap()
    w2_d = nc.dram_tensor("w2", [FCH, P, C], F8, kind="ExternalInput").ap()
    rowq_d = nc.dram_tensor("rowq", [1, DPAD], F16, kind="ExternalInput").ap()
    rowk_d = nc.dram_tensor("rowk", [1, DPAD], F16, kind="ExternalInput").ap()
    rowv_d = nc.dram_tensor("rowv", [1, C], F16, kind="ExternalInput").ap()
    rowp_d = nc.dram_tensor("rowp", [1, C], F16, kind="ExternalInput").ap()
    rowl_d = nc.dram_tensor("rowl", [1, C], F16, kind="ExternalInput").ap()
    b1t_d = nc.dram_tensor("b1t", [P, FCH], F32, kind="ExternalInput").ap()
    mask_d = nc.dram_tensor("maskmul", [P, 2 * P], F16, kind="ExternalInput").ap()
    out_d = nc.dram_tensor("out", [C, TOK], F16, kind="ExternalOutput").ap()

    with tile.TileContext(nc) as tc:
        with nc.allow_low_precision(reason="f16 activations within tolerance"):
            _emit(nc, tc, flags, xt_d, wq_d, wk_d, wv_d, wp_d, w1_d, w2_d,
                  rowq_d, rowk_d, rowv_d, rowp_d, rowl_d, b1t_d, mask_d,
                  out_d)
    nc.compile()
    return nc


def _emit(nc, tc, flags, xt_d, wq_d, wk_d, wv_d, wp_d, w1_d, w2_d,
          rowq_d, rowk_d, rowv_d, rowp_d, rowl_d, b1t_d, mask_d,
          out_d):
    from contextlib import ExitStack
    with ExitStack() as ctx:
        const = ctx.enter_context(tc.tile_pool(name="const", bufs=1))
        ln = ctx.enter_context(tc.tile_pool(name="ln", bufs=2))
        grp = ctx.enter_context(tc.tile_pool(name="grp", bufs=2))
        att = ctx.enter_context(tc.tile_pool(name="att", bufs=2))
        outp = ctx.enter_context(tc.tile_pool(name="outp", bufs=2))
        psum = ctx.enter_context(tc.tile_pool(name="psum", bufs=2, space="PSUM"))
        spp = ctx.enter_context(tc.tile_pool(name="spp", bufs=2, space="PSUM"))
        stp = ctx.enter_context(tc.tile_pool(name="stp", bufs=2, space="PSUM"))

        nc.gpsimd.load_library(library_config.proxy)

        # ---- constants ----
        wq_sb = const.tile([P, CCH, DPAD], F8)
        wk_sb = const.tile([P, CCH, DPAD], F8)
        wv_sb = const.tile([P, CCH, C], F8)
        wp_sb = const.tile([P, QMT, C], F8)
        w1_sb = const.tile([P, CCH, FF], F8)
        w2_sb = const.tile([P, FCH, C], F8)

        # group 0's x chunks first so stats of group 0 aren't stuck
        # behind ~4 MB of weight DMA at kernel start
        st = [dict() for _ in range(NG)]
        for g0 in range(2):
            xTg = grp.tile([P, CCH, GT], F16, tag="xT", name="xT")
            st[g0]["xT"] = xTg
            for cc in range(CCH):
                nc.sync.dma_start(xTg[:, cc, :],
                                  xt_d[cc * P:(cc + 1) * P,
                                       g0 * GT:(g0 + 1) * GT])

        for cc in range(CCH):
            nc.sync.dma_start(wq_sb[:, cc, :], wq_d[cc])
            nc.sync.dma_start(wk_sb[:, cc, :], wk_d[cc])
            nc.sync.dma_start(wv_sb[:, cc, :], wv_d[cc])
            nc.sync.dma_start(w1_sb[:, cc, :], w1_d[cc])
        for m in range(QMT):
            nc.sync.dma_start(wp_sb[:, m, :], wp_d[m])
        for fc in range(FCH):
            nc.sync.dma_start(w2_sb[:, fc, :], w2_d[fc])
        mask_sb = const.tile([P, 2 * P], F16)
        nc.sync.dma_start(mask_sb, mask_d)
        mask3_sb = mask_sb.rearrange("p (b c) -> p b c", c=P)

        ones_sb = const.tile([1, GT], F16)
        nc.vector.memset(ones_sb, 1.0)
        onesc_sb = const.tile([P, 1], F16)   # 1/C column (stats stationary)
        nc.vector.memset(onesc_sb, RC)

        # preload the Sqrt activation table while the weight DMAs are in
        # flight, so group 0's LN chain doesn't pay the swap
        warm = ln.tile([1, GT], F32, tag="warm")
        nc.scalar.activation(out=warm, in_=ones_sb,
                             func=mybir.ActivationFunctionType.Sqrt)

        rowq_sb = const.tile([1, DPAD], F16)
        rowk_sb = const.tile([1, DPAD], F16)
        rowv_sb = const.tile([1, C], F16)
        rowp_sb = const.tile([1, C], F16)
        rowl_sb = const.tile([1, C], F16)
        b1t_sb = const.tile([P, FCH], F32)
        if flags["rowq"]:
            nc.sync.dma_start(rowq_sb, rowq_d)
        if flags["rowk"]:
            nc.sync.dma_start(rowk_sb, rowk_d)
        if flags["rowv"]:
            nc.sync.dma_start(rowv_sb, rowv_d)
        if flags["rowp"]:
            nc.sync.dma_start(rowp_sb, rowp_d)
        if flags["rowl"]:
            nc.sync.dma_start(rowl_sb, rowl_d)
        if flags["b1t"]:
            nc.sync.dma_start(b1t_sb, b1t_d)

        def stats_mms(ps_st, srcs):
            """6 stats matmuls: psum row 0 = mean row [1, GT], psum row 32 =
            E[x^2] row.  srcs = [(x_chunk, xsq_chunk)] * CCH."""
            # all x-row matmuls first: they don't depend on the GPSIMD
            # square chain, so the PE starts while xsq is still computing
            for cc in range(CCH):
                nc.tensor.matmul(ps_st[0:1, :], lhsT=onesc_sb, rhs=srcs[cc][0],
                                 start=(cc == 0), stop=(cc == CCH - 1))
            for cc in range(CCH):
                nc.tensor.matmul(ps_st[32:33, :], lhsT=onesc_sb,
                                 rhs=srcs[cc][1],
                                 start=(cc == 0), stop=(cc == CCH - 1))

        def ln_rows(ps_st, tag):
            """mean row (psum p0) + E[x^2] row (psum p32) ->
            (rstd f16 [1,GT], mean*rstd f16 [1,GT]) in SBUF."""
            # mean row to SBUF once (DVE), squared on GPSIMD -- keeps the
            # Square activation (and its table swap) off the busy ACT queue
            mrow = ln.tile([1, GT], F32, tag=f"mrow{tag}")
            nc.vector.tensor_copy(mrow, ps_st[0:1, :])
            musq = ln.tile([1, GT], F32, tag=f"musq{tag}")
            nc.vector.tensor_mul(musq, mrow, mrow)
            var = ln.tile([1, GT], F32, tag=f"var{tag}")
            nc.vector.tensor_sub(var, ps_st[32:33, :], musq)
            rvar = ln.tile([1, GT], F32, tag=f"rvar{tag}")
            nc.vector.reciprocal_approx_fast(out=rvar, in_=var)
            rstd = ln.tile([1, GT], F16, tag=f"rstd{tag}")
            nc.scalar.activation(out=rstd, in_=rvar,
                                 func=mybir.ActivationFunctionType.Sqrt)
            mr = ln.tile([1, GT], F16, tag=f"mr{tag}")
            nc.vector.tensor_mul(mr, mrow, rstd)
            return rstd, mr

        def ln_apply(src3, rstd_b, mr_b, dst3, tag):
            # (x - mu) * rstd = x * rstd_b - (mu * rstd)_b ; dst is fp8
            for cc in range(CCH):
                t1 = ln.tile([P, GT], F16, tag=f"t1{tag}")
                nc.vector.tensor_mul(t1, src3[:, cc, :], rstd_b)
                nc.vector.tensor_sub(dst3[:, cc, :], t1, mr_b)

        def ln_finish(g, which, src_key, dst_tag):
            s = st[g]
            rstd, mr = ln_rows(s["st" + which], which)
            rstd_b = ln.tile([P, GT], F16, tag=f"rstdb{which}")
            mr_b = ln.tile([P, GT], F16, tag=f"mrb{which}")
            nc.gpsimd.partition_broadcast(rstd_b, rstd)
            nc.gpsimd.partition_broadcast(mr_b, mr)
            dst3 = grp.tile([P, CCH, GT], F8, tag=dst_tag, name=dst_tag)
            ln_apply(s[src_key], rstd_b, mr_b, dst3, which)
            return dst3

        DR = mybir.MatmulPerfMode.DoubleRow

        def contract3(ps_out, lhs3, rhs3, lcols, stop=True):
            """K=384 contraction as DoubleRow(chunks 0-1) + plain fp8
            (chunk 2) in one accumulation group.  lhs3/rhs3 are
            [P, CCH, *] fp8 tiles; lcols slices the lhsT free dim."""
            nc.tensor.matmul(ps_out, lhsT=lhs3[:, 0:2, lcols],
                             rhs=rhs3[:, 0:2, :],
                             start=True, stop=False, perf_mode=DR)
            nc.tensor.matmul(ps_out, lhsT=lhs3[:, 2, lcols],
                             rhs=rhs3[:, 2, :],
                             start=False, stop=stop)

        # ============ stage A: load x + LN1 stats for group g ============
        def emit_ln1_stats(g):
            s = st[g]
            if "xT" not in s:
                xT = grp.tile([P, CCH, GT], F16, tag="xT", name="xT")
                s["xT"] = xT
                for cc in range(CCH):
                    nc.sync.dma_start(xT[:, cc, :],
                                      xt_d[cc * P:(cc + 1) * P,
                                           g * GT:(g + 1) * GT])
            xT = s["xT"]
            ps_st = stp.tile([P, GT], F32, tag="st")
            s["st1"] = ps_st
            srcs = []
            for cc in range(CCH):
                xsq = ln.tile([P, GT], F16, tag="xsq")
                nc.gpsimd.tensor_mul(xsq, xT[:, cc, :], xT[:, cc, :])
                srcs.append((xT[:, cc, :], xsq))
            stats_mms(ps_st, srcs)

        # ============ LN1 finish -> hT ; LN2 finish -> h2T ============
        def emit_ln1_post(g):
            st[g]["hT"] = ln_finish(g, "1", "xT", "hT")

        def emit_ln2_post(g):
            s = st[g]
            s["h2T"] = ln_finish(g, "2", "x1T", "h2T")
            s["rg"] = grp.tile([P, FCH, GT], F8, tag="rg", name="rg")

        # ============ stage B: QKV for group g ============
        def emit_qkv(g):
            s = st[g]
            hT = s["hT"]
            qT = grp.tile([P, QMT, GT], F16, tag="qT")
            kT = grp.tile([P, QMT, GT], F16, tag="kT")
            vaug = grp.tile([P, GTT, VW], F16, tag="vaug")
            s["qT"], s["kT"], s["vaug"] = qT, kT, vaug
            # V first: at the iteration boundary its psum banks come off the
            # same rotation but nothing downstream is waiting on V yet, so a
            # late bank release can't stall the scores
            for stt in range(GTT):
                ps = psum.tile([P, GT], F32, tag="ps", name="ps")
                contract3(ps[:, :C], hT, wv_sb,
                          slice(stt * P, (stt + 1) * P),
                          stop=not flags["rowv"])
                if flags["rowv"]:
                    nc.tensor.matmul(ps[:, :C], lhsT=ones_sb[:, :P], rhs=rowv_sb,
                                     start=False, stop=True)
                v4 = vaug[:, stt, :].rearrange("p (q w) -> p q w", w=PW)
                p8 = ps[:, :C].rearrange("p (h w) -> p h w", w=HS)
                nc.scalar.mul(v4[:, :, 1:HS + 1], p8[:, 0::2, :], RSCL)
                nc.scalar.mul(v4[:, :, 65:65 + HS], p8[:, 1::2, :], RSCL)
                nc.vector.memset(v4[:, :, 0], 1.0)
                nc.vector.memset(v4[:, :, 64], 1.0)
                nc.vector.memset(v4[:, :, HS + 1:64], 0.0)
                nc.vector.memset(v4[:, :, 65 + HS:PW], 0.0)
            for dst, w_sb, row_sb, rowf, on_act in (
                    (qT, wq_sb, rowq_sb, flags["rowq"], True),
                    (kT, wk_sb, rowk_sb, flags["rowk"], False)):
                for m in range(QMT):
                    ps = psum.tile([P, GT], F32, tag="ps", name="ps")
                    contract3(ps, w_sb, hT, slice(m * P, (m + 1) * P),
                              stop=not rowf)
                    if rowf:
                        nc.tensor.matmul(ps, lhsT=row_sb[:, m * P:(m + 1) * P],
                                         rhs=ones_sb, start=False, stop=True)
                    # split the psum->SBUF copies across ACT and DVE so the
                    # copy chain keeps pace with the QKV matmul stream
                    if on_act:
                        nc.scalar.copy(dst[:, m, :], ps)
                    else:
                        nc.vector.tensor_copy(dst[:, m, :], ps)

        # ============ attention closures for group g ============
        def attn_closures(g):
            s = st[g]
            qT, kT, vaug = s["qT"], s["kT"], s["vaug"]
            attnT = grp.tile([P, QMT, GT], F8, tag="attnT", name="attnT")
            s["attnT"] = attnT
            pair_ps = [None] * NPAIR
            ewss = [None] * NPAIR
            rec_bs = [None] * NPAIR

            def sc(m):
                """score MMs + exp + causal mask for pair m, both seqs.
                MMs alternate head row offsets so next LDWEIGHTS overlaps."""
                ews = []
                for b2 in range(2):
                    s0 = b2 * T
                    sps = [spp.tile([P, T + P], F32, tag="sp", name="sp")
                           for _ in range(2)]
                    for hh in range(2):
                        off = 64 * hh
                        nc.tensor.matmul(sps[hh][:, :T],
                                         lhsT=kT[off:off + HS, m, s0:s0 + P],
                                         rhs=qT[off:off + HS, m, s0:s0 + T],
                                         start=True, stop=False)
                    for hh in range(2):
                        off = 64 * hh
                        nc.tensor.matmul(
                            sps[hh][:, T:T + P],
                            lhsT=kT[off:off + HS, m, s0 + P:s0 + T],
                            rhs=qT[off:off + HS, m, s0 + P:s0 + T],
                            start=False, stop=True)
                    for hh in range(2):
                        ew = att.tile([P, T + P], F16, tag="ew", name="ew",
                                      bufs=12)
                        nc.scalar.activation(
                            out=ew, in_=sps[hh],
                            func=mybir.ActivationFunctionType.Exp,
                            scale=ISCALE / (SCL * SCL))
                        # causal mask on the two diagonal blocks only
                        ew3 = ew.rearrange("p (b c) -> p b c", c=P)[:, 0::2, :]
                        if hh == 0:
                            nc.vector.tensor_mul(ew3, ew3, mask3_sb)
                        else:
                            nc.gpsimd.tensor_mul(ew3, ew3, mask3_sb)
                        ews.append(ew)
                ewss[m] = ews

            def av(m):
                """attn^T += V^T scores^T; den rows 0/64 -> reciprocal ->
                GPSIMD partition broadcast."""
                pp = psum.tile([P, GT], F32, tag="pp", name="pp")
                pair_ps[m] = pp
                c0 = m * PW
                for b2 in range(2):
                    tb = b2 * T
                    for hh in range(2):
                        ew = ewss[m][b2 * 2 + hh]
                        r0 = 64 * hh
                        ca = c0 + 64 * hh
                        nc.tensor.matmul(pp[r0:r0 + 64, tb:tb + T],
                                         lhsT=vaug[:, b2 * 2, ca:ca + 64],
                                         rhs=ew[:, :T],
                                         start=True, stop=False)
                        nc.tensor.matmul(pp[r0:r0 + 64, tb + P:tb + T],
                                         lhsT=vaug[:, b2 * 2 + 1, ca:ca + 64],
                                         rhs=ew[:, T:T + P],
                                         start=False, stop=True)
                # den rows: 0 (head a), 64 (head b).  Partition-aligned
                # reciprocal (rows stay in their own partitions), DMA moves
                # row 64 down to a partition-0 row (GPSIMD pb can only read
                # partition 0 and write partitions [0, channels)), then two
                # full-height broadcasts so the fin muls read same-base
                # operands.
                recAB = att.tile([65, GT], F32, tag="recAB", name="recAB")
                nc.vector.reciprocal_approx_fast(out=recAB, in_=pp[0:65, :])
                recB = att.tile([1, GT], F32, tag="recB", name="recB")
                nc.sync.dma_start(recB, recAB[64:65, :])
                rec_bL = att.tile([P, GT], F32, tag="recbL", name="recbL")
                rec_bH = att.tile([P, GT], F32, tag="recbH", name="recbH")
                nc.gpsimd.partition_broadcast(rec_bL, recAB[0:1, :])
                nc.gpsimd.partition_broadcast(rec_bH, recB)
                rec_bs[m] = (rec_bL, rec_bH)

            def fin(m):
                rec_bL, rec_bH = rec_bs[m]
                nc.vector.tensor_mul(attnT[0:64, m, :], pair_ps[m][0:64, :],
                                     rec_bL[0:64, :])
                nc.vector.tensor_mul(attnT[64:P, m, :], pair_ps[m][64:P, :],
                                     rec_bH[64:P, :])

            return sc, av, fin

        # ============ FFN chunks for group g ============
        def w1_chunk(g, fcs):
            s = st[g]
            h2T, rg = s["h2T"], s["rg"]
            for fc in fcs:
                ps = psum.tile([P, GT], F32, tag="ps", name="ps")
                contract3(ps, w1_sb, h2T, slice(fc * P, (fc + 1) * P))
                # psum holds SCL*(h2 @ W1); relu is positively homogeneous
                if flags["b1t"] or fc % 2 == 0:
                    nc.scalar.activation(
                        out=rg[:, fc, :], in_=ps,
                        func=mybir.ActivationFunctionType.Relu,
                        bias=(b1t_sb[:, fc:fc + 1] if flags["b1t"] else 0.0),
                        scale=RSCL)
                else:
                    nc.vector.tensor_scalar(
                        out=rg[:, fc, :], in0=ps,
                        scalar1=RSCL, scalar2=0.0,
                        op0=mybir.AluOpType.mult,
                        op1=mybir.AluOpType.max)

        def w2_chunk(g, oc):
            s = st[g]
            ps = psum.tile([P, GT], F32, tag="ps", name="ps")
            ocs = slice(oc * P, (oc + 1) * P)
            for p2 in range(FCH // 2):
                nc.tensor.matmul(ps, lhsT=w2_sb[:, 2 * p2:2 * p2 + 2, ocs],
                                 rhs=s["rg"][:, 2 * p2:2 * p2 + 2, :],
                                 start=(p2 == 0),
                                 stop=(p2 == FCH // 2 - 1 and not flags["rowl"]),
                                 perf_mode=DR)
            if flags["rowl"]:
                nc.tensor.matmul(ps, lhsT=rowl_sb[:, ocs],
                                 rhs=ones_sb, start=False, stop=True)
            ot = outp.tile([P, GT], F16, tag="ot", name="ot")
            nc.vector.scalar_tensor_tensor(
                out=ot, in0=ps, scalar=RSCL, in1=s["x1T"][:, oc, :],
                op0=mybir.AluOpType.mult, op1=mybir.AluOpType.add)
            nc.sync.dma_start(out_d[oc * P:(oc + 1) * P,
                                    g * GT:(g + 1) * GT], ot)

        # ======= proj + residual + LN2 stats for group g =======
        def emit_proj_ln2stats(g):
            s = st[g]
            attnT = s["attnT"]
            x1T = grp.tile([P, CCH, GT], F16, tag="x1T")
            s["x1T"] = x1T
            ps_st = stp.tile([P, GT], F32, tag="st")
            s["st2"] = ps_st
            srcs = []
            for oc in range(CCH):
                # pp tag: its previous readers (reciprocal + fin muls) are
                # done by now, unlike the ps tag whose release would gate the
                # next group's first QKV matmuls on this iteration's DVE tail
                ps = psum.tile([P, GT], F32, tag="pp", name="pp")
                ocs = slice(oc * P, (oc + 1) * P)
                for p2 in range(QMT // 2):
                    nc.tensor.matmul(ps, lhsT=wp_sb[:, 2 * p2:2 * p2 + 2, ocs],
                                     rhs=attnT[:, 2 * p2:2 * p2 + 2, :],
                                     start=(p2 == 0),
                                     stop=(p2 == QMT // 2 - 1
                                           and not flags["rowp"]),
                                     perf_mode=DR)
                if flags["rowp"]:
                    nc.tensor.matmul(ps, lhsT=rowp_sb[:, ocs],
                                     rhs=ones_sb, start=False, stop=True)
                nc.vector.scalar_tensor_tensor(
                    out=x1T[:, oc, :], in0=ps, scalar=RSCL,
                    in1=s["xT"][:, oc, :],
                    op0=mybir.AluOpType.mult, op1=mybir.AluOpType.add)
                xsq = ln.tile([P, GT], F16, tag="xsq2")
                nc.gpsimd.tensor_mul(xsq, x1T[:, oc, :], x1T[:, oc, :])
                srcs.append((x1T[:, oc, :], xsq))
            stats_mms(ps_st, srcs)

        # ============ software pipeline ============
        # Iteration gi: ln1-stats(gi) | ln2-post(gi-2) | qkv(gi-1) |
        # ln1-post(gi) | attn(gi-1) interleaved with ffn(gi-2) |
        # proj+ln2-stats(gi-1).
        FSPLIT = [(0, 1, 2), (3, 4, 5), (6, 7, 8), (9, 10, 11)]
        for gi in range(NG + 2):
            if gi < NG:
                emit_ln1_stats(gi)
            if 1 <= gi <= NG:
                emit_qkv(gi - 1)
            if 2 <= gi:
                emit_ln2_post(gi - 2)
            if gi < NG:
                emit_ln1_post(gi)
            a = gi - 1 if 1 <= gi <= NG else None
            f = gi - 2 if gi >= 2 else None
            if a is not None:
                sc, av, fin = attn_closures(a)
            if a is not None and f is not None:
                sc(0); w1_chunk(f, FSPLIT[0]); av(0)
                sc(1); w1_chunk(f, FSPLIT[1]); av(1); fin(0)
                sc(2); w1_chunk(f, FSPLIT[2]); av(2); fin(1)
                sc(3); w1_chunk(f, FSPLIT[3]); av(3); fin(2)
                fin(3)
                for oc in range(CCH):
                    w2_chunk(f, oc)
                st[f].clear()
            elif a is not None:
                sc(0); av(0)
                sc(1); av(1); fin(0)
                sc(2); av(2); fin(1)
                sc(3); av(3); fin(2)
                fin(3)
            elif f is not None:
                for fcs in FSPLIT:
                    w1_chunk(f, fcs)
                for oc in range(CCH):
                    w2_chunk(f, oc)
                st[f].clear()
            if 1 <= gi <= NG:
                emit_proj_ln2stats(gi - 1)


def _q8(a):
    return np.clip(a * SCL, -240.0, 240.0).astype(NPF8)


def _prep_weights(Wq, Wk, Wv, Wproj, bproj, W1, b1, W2, b2, g1, beta1, g2,
                  beta2):
    f16 = np.float16
    g1 = g1.astype(np.float64)
    g2 = g2.astype(np.float64)

    def qk_pack(W):
        Ws = g1[None, :, None] * W.astype(np.float64)      # [H, C, HS]
        pad = np.zeros((CCH, P, DPAD), np.float64)
        row = np.zeros((1, DPAD), np.float64)
        beta_r = np.einsum('c,hcd->hd', beta1.astype(np.float64),
                           W.astype(np.float64))
        for h in range(H):
            m, hh = divmod(h, 2)
            col = m * P + 64 * hh
            pad[:, :, col:col + HS] = Ws[h].reshape(CCH, P, HS)
            row[0, col:col + HS] = beta_r[h]
        # weights fp8 at SCL x; the beta row rides the f16 bias matmul and
        # must carry the same scale so the psum stays uniformly SCL x q
        return _q8(pad), (row * SCL).astype(f16)

    wq_pad, rowq = qk_pack(Wq)
    wk_pad, rowk = qk_pack(Wk)

    # V: concat-head layout [C, C]
    Wvs = (g1[None, :, None] * Wv.astype(np.float64))       # [H, C, HS]
    wv = np.transpose(Wvs, (1, 0, 2)).reshape(C, C)         # [c, h*HS+d]
    beta_v = np.einsum('c,hcd->hd', beta1.astype(np.float64),
                       Wv.astype(np.float64)).reshape(1, C)
    wv = _q8(wv.reshape(CCH, P, C))
    rowv = (beta_v * SCL).astype(f16)

    # proj packed to the attn^T pair-row layout: chunk m holds heads
    # 2m (rows 1:49) and 2m+1 (rows 65:113); den/junk rows zero.
    Wp3 = Wproj.astype(np.float64).reshape(H, HS, C)
    wp = np.zeros((QMT, P, C), np.float64)
    for h in range(H):
        m, hh = divmod(h, 2)
        r0 = 1 if hh == 0 else 65
        wp[m, r0:r0 + HS, :] = Wp3[h]
    wp = _q8(wp)
    rowp = (bproj.astype(np.float64) * SCL).astype(f16).reshape(1, C)

    W1s = g2[:, None] * W1.astype(np.float64)
    w1p = _q8(W1s.reshape(CCH, P, FF))
    b1tot = (b1.astype(np.float64)
             + beta2.astype(np.float64) @ W1.astype(np.float64))
    b1t = b1tot.astype(np.float32).reshape(FCH, P).T.copy()   # [P, FCH]

    w2p = _q8(W2.astype(np.float64).reshape(FCH, P, C))
    rowl = (b2.astype(np.float64) * SCL).astype(f16).reshape(1, C)

    tri = np.triu(np.ones((P, P), np.float64))  # [s, t]: valid iff s <= t
    maskmul = np.concatenate([tri, tri], axis=1).astype(f16)

    wdict = dict(wq=wq_pad, wk=wk_pad, wv=wv, wp=wp, w1=w1p, w2=w2p,
                 rowq=rowq, rowk=rowk, rowv=rowv, rowp=rowp, rowl=rowl,
                 b1t=b1t, maskmul=maskmul)
    flags = {k: bool(np.any(wdict[k] != 0))
             for k in ("rowq", "rowk", "rowv", "rowp", "rowl", "b1t")}
    return wdict, flags


_CACHED = {}


def _get_program(flags):
    key = tuple(sorted(flags.items()))
    if key not in _CACHED:
        _CACHED[key] = _build_program(flags)
    return _CACHED[key]


def _run(inputs, trace=False):
    x = np.asarray(inputs["x"], np.float32)
    wdict, flags = _prep_weights(
        np.asarray(inputs["Wq"]), np.asarray(inputs["Wk"]),
        np.asarray(inputs["Wv"]), np.asarray(inputs["Wproj"]),
        np.asarray(inputs["bproj"]), np.asarray(inputs["W1"]),
        np.asarray(inputs["b1"]), np.asarray(inputs["W2"]),
        np.asarray(inputs["b2"]), np.asarray(inputs["g1"]),
        np.asarray(inputs["beta1"]), np.asarray(inputs["g2"]),
        np.asarray(inputs["beta2"]))

    shards = x.reshape(NCORES, TOK, C)
    in_maps = [dict(wdict,
                    xt=np.ascontiguousarray(
                        shards[i].T.astype(np.float16)))
               for i in range(NCORES)]
    nc = _get_program(flags)
    res = run_bass_kernel_spmd(nc, in_maps, list(range(NCORES)), trace=trace)
    out = np.stack([np.ascontiguousarray(res.results[i]["out"].T)
                    for i in range(NCORES)])
    return out.reshape(B, T, C).astype(np.float32), res


def kernel(**inputs):
    out, _ = _run(inputs, trace=False)
    return out
